# revision 45
# baseline (speedup 1.0000x reference)
"""Multi-head attention (B=2, S=2048, D=512, H=8) on 8 trn2 NeuronCores.

Sharding: data-parallel over batch (2) x tensor-parallel over head-pairs (4).
Core c handles batch c//4 and heads [2*(c%4), 2*(c%4)+1] (128 model dims).

Device kernel (SPMD, identical program, per-core inputs):
  inputs:  xqT/xkT/xvT [512,2048] (host-pretransposed), wq/wk/wv [512,128]
           (column slice), wo [128,512] (row slice), bq/bk [128,1]
  outputs: attn_out [2,2048,2048] (this core's two heads, softmaxed),
           out_partial [2048,512] (this core's contribution to out)

Host folds the v/o biases in afterwards: since each softmax row sums to 1,
ctx = attn@(vh + 1*bv^T) = attn@vh + bv, so out += bv @ wo_w + wo_b.
"""

import numpy as np

import concourse.bass as bass
import concourse.mybir as mybir
from concourse import bacc
from concourse.tile import TileContext
from concourse.bass_utils import run_bass_kernel_spmd

B, S, D = 2, 2048, 512
H, DEP = 8, 64
N_CORES = 8
HPC = 2          # heads per core
D2 = HPC * DEP   # 128 model dims per core
NT = S // 128    # 16 tiles of 128 rows
F32 = mybir.dt.float32
F16 = mybir.dt.float16

_CACHED_NC = None


def _build_nc():
    nc = bacc.Bacc(None, target_bir_lowering=False, debug=False)

    xqT = nc.declare_dram_parameter("xqT", [D, S], F16, isOutput=False)
    xkT = nc.declare_dram_parameter("xkT", [D, S], F16, isOutput=False)
    xvT = nc.declare_dram_parameter("xvT", [D, S], F16, isOutput=False)
    wq = nc.declare_dram_parameter("wq", [D, D2], F16, isOutput=False)
    wk = nc.declare_dram_parameter("wk", [D, D2], F16, isOutput=False)
    wv = nc.declare_dram_parameter("wv", [D, D2], F16, isOutput=False)
    wo = nc.declare_dram_parameter("wo", [D2, D], F16, isOutput=False)
    bq = nc.declare_dram_parameter("bq", [D2, 1], F32, isOutput=False)
    bk = nc.declare_dram_parameter("bk", [D2, 1], F32, isOutput=False)
    attn_out = nc.declare_dram_parameter("attn_out", [HPC, S, S], F32, isOutput=True)
    out_partial = nc.declare_dram_parameter("out_partial", [S, D], F32, isOutput=True)

    scale = 1.0 / np.sqrt(DEP)
    EXP = mybir.ActivationFunctionType.Exp

    with TileContext(nc) as tc:
        with tc.tile_pool(name="singles", bufs=1) as singles:
            # persistent SBUF tensors
            qhT = singles.tile([D2, S], F16, tag="qhT")    # [128(2h x 64d), 2048]
            khT = singles.tile([D2, S], F16, tag="khT")
            vh = singles.tile([128, S], F16, tag="vh")     # block st: [128(k),128(d)]
            ctxT = singles.tile([D2, S], F16, tag="ctxT")  # [128(d), 2048(q)] unnorm
            recip = singles.tile([128, HPC * NT], F32, tag="recip")  # col h*16+t
            x_v = singles.tile([128, 4, S], F16, tag="x_v")
            wq_sb = singles.tile([128, 4, D2], F16, tag="wq")
            wk_sb = singles.tile([128, 4, D2], F16, tag="wk")
            wv_sb = singles.tile([128, 4, D2], F16, tag="wv")
            wo_sb = singles.tile([D2, D], F16, tag="wo")
            bq_sb = singles.tile([D2, 1], F32, tag="bq")
            bk_sb = singles.tile([D2, 1], F32, tag="bk")

            nc.sync.dma_start(out=wq_sb[:], in_=wq.ap().rearrange("(c p) d -> p c d", p=128))
            nc.sync.dma_start(out=wk_sb[:], in_=wk.ap().rearrange("(c p) d -> p c d", p=128))
            nc.sync.dma_start(out=wv_sb[:], in_=wv.ap().rearrange("(c p) d -> p c d", p=128))
            nc.sync.dma_start(out=wo_sb[:], in_=wo.ap())
            nc.sync.dma_start(out=bq_sb[:], in_=bq.ap())
            nc.sync.dma_start(out=bk_sb[:], in_=bk.ap())

            # ---------------- stage 1: q/k projections ----------------
            # (v is loaded here but projected inside phase A)
            with (
                tc.tile_pool(name="xpool", bufs=2) as xpool,
                tc.tile_pool(name="psum1", bufs=2, space="PSUM") as psum1,
            ):
                for xT, w_sb, b_sb, outT in (
                    (xkT, wk_sb, bk_sb, khT),
                    (xqT, wq_sb, bq_sb, qhT),
                ):
                    x_sb = xpool.tile([128, 4, S], F16, tag="x")
                    nc.sync.dma_start(
                        out=x_sb[:], in_=xT.ap().rearrange("(c p) s -> p c s", p=128)
                    )
                    for ncx in range(4):
                        ps = psum1.tile([D2, 512], F32, tag="ps1")
                        for cc in range(4):
                            nc.tensor.matmul(
                                ps[:],
                                w_sb[:, cc, :],
                                x_sb[:, cc, ncx * 512 : (ncx + 1) * 512],
                                start=(cc == 0),
                                stop=(cc == 3),
                            )
                        nc.vector.tensor_scalar_add(
                            outT[:, ncx * 512 : (ncx + 1) * 512], ps[:], b_sb[:]
                        )
                nc.sync.dma_start(
                    out=x_v[:], in_=xvT.ap().rearrange("(c p) s -> p c s", p=128)
                )

            # ---------------- stage 2: attention ----------------
            with (
                tc.tile_pool(name="psum_s", bufs=3, space="PSUM") as pool_s,
                tc.tile_pool(name="psum_av", bufs=2, space="PSUM") as pool_av,
                tc.tile_pool(name="Ppool", bufs=3) as Ppool,
                tc.tile_pool(name="PTpool", bufs=2) as PTpool,
                tc.tile_pool(name="sumpool", bufs=4) as sumpool,
                tc.tile_pool(name="opool", bufs=2) as opool,
            ):
                PTs = {}

                def vh_group(g):
                    """Project v s-tiles 4g..4g+3 into vh (dense 16-MM burst)."""
                    ps_v = pool_av.tile([128, 512], F32, tag="av", name="ps_v")
                    for i in range(4):
                        st = 4 * g + i
                        for cc in range(4):
                            nc.tensor.matmul(
                                ps_v[:, i * 128 : (i + 1) * 128],
                                x_v[:, cc, st * 128 : (st + 1) * 128],
                                wv_sb[:, cc, :],
                                start=(cc == 0),
                                stop=(cc == 3),
                            )
                    nc.vector.tensor_copy(
                        out=vh[:, 4 * g * 128 : (4 * g + 4) * 128], in_=ps_v[:]
                    )

                def av_part(hT, qc, c):
                    """8-deep AV chain (kt=c*8..c*8+7) folded into ctxT."""
                    ps_av = pool_av.tile([DEP, 512], F32, tag="av", name="ps_av")
                    PTh = PTs[hT]
                    for j in range(8):
                        kt = c * 8 + j
                        nc.tensor.matmul(
                            ps_av[:],
                            vh[:, kt * 128 + hT * DEP : kt * 128 + (hT + 1) * DEP],
                            PTh[:, kt, qc * 512 : (qc + 1) * 512],
                            start=(j == 0),
                            stop=(j == 7),
                        )
                    ct = ctxT[hT * DEP : (hT + 1) * DEP, qc * 512 : (qc + 1) * 512]
                    if c == 0:
                        nc.vector.tensor_copy(out=ct, in_=ps_av[:])
                    else:
                        nc.vector.tensor_add(ct, ct, ps_av[:])

                for phase in range(2):
                    hP, hT = phase, 1 - phase
                    hsP = slice(hP * DEP, (hP + 1) * DEP)
                    hsT = slice(hT * DEP, (hT + 1) * DEP)
                    PTs[hT] = PTpool.tile([128, NT, S], F16, tag="PT", name="PT")
                    for t in range(NT):
                        # ---- PT stream (head hT, k-tile t) ----
                        for half in range(2):
                            ps_t = pool_s.tile([128, 1024], F32, tag="s", name="ps_t")
                            for ncx in range(2):
                                nc.tensor.matmul(
                                    ps_t[:, ncx * 512 : (ncx + 1) * 512],
                                    khT[hsT, t * 128 : (t + 1) * 128],
                                    qhT[hsT, half * 1024 + ncx * 512 : half * 1024 + (ncx + 1) * 512],
                                    start=True,
                                    stop=True,
                                )
                            nc.scalar.activation(
                                PTs[hT][:, t, half * 1024 : (half + 1) * 1024],
                                ps_t[:],
                                EXP,
                                scale=float(scale),
                            )
                        # ---- P stream (head hP, q-tile t), other row-group ----
                        P_sb = Ppool.tile([128, S], F32, tag="P")
                        sums = sumpool.tile([128, 1], F32, tag="sums")
                        for half in range(2):
                            ps_p = pool_s.tile([128, 1024], F32, tag="s", name="ps_p")
                            for ncx in range(2):
                                nc.tensor.matmul(
                                    ps_p[:, ncx * 512 : (ncx + 1) * 512],
                                    qhT[hsP, t * 128 : (t + 1) * 128],
                                    khT[hsP, half * 1024 + ncx * 512 : half * 1024 + (ncx + 1) * 512],
                                    start=True,
                                    stop=True,
                                )
                            nc.vector.tensor_copy(
                                out=P_sb[:, half * 1024 : (half + 1) * 1024],
                                in_=ps_p[:],
                            )
                        nc.scalar.activation(
                            P_sb[:], P_sb[:], EXP,
                            scale=float(scale), accum_out=sums[:],
                        )
                        rc = recip[:, hP * NT + t : hP * NT + t + 1]
                        nc.vector.reciprocal(rc, sums[:])
                        nc.vector.tensor_scalar_mul(P_sb[:], P_sb[:], rc)
                        nc.sync.dma_start(
                            out=attn_out[hP, t * 128 : (t + 1) * 128, :], in_=P_sb[:]
                        )
                        # ---- dense PE interleaves ----
                        if phase == 0 and t % 4 == 1:
                            vh_group(t // 4)          # t=1,5,9,13
                        if phase == 0 and t in (9, 11, 13, 15):
                            av_part(1, (t - 9) // 2, 0)
                        elif phase == 1 and t in (1, 3, 5, 7):
                            av_part(1, (t - 1) // 2, 1)
                        elif phase == 1 and t in (9, 11, 13, 15):
                            av_part(0, (t - 9) // 2, 0)

                # ---- tail: AV(h0) second half + output projection ----
                for t in range(NT):
                    if t % 4 == 0:
                        av_part(0, t // 4, 1)
                    ps_o = pool_s.tile([128, 1024], F32, tag="s", name="ps_o")
                    s1 = ps_o[:, 0:512]
                    s0 = ps_o[:, 512:1024]
                    nc.tensor.matmul(
                        s1, ctxT[DEP:, t * 128 : (t + 1) * 128], wo_sb[DEP:, :],
                        start=True, stop=True,
                    )
                    nc.tensor.matmul(
                        s0, ctxT[:DEP, t * 128 : (t + 1) * 128], wo_sb[:DEP, :],
                        start=True, stop=True,
                    )
                    o1 = opool.tile([128, D], F32, tag="o1")
                    nc.scalar.activation(
                        o1[:], s1, mybir.ActivationFunctionType.Copy,
                        scale=recip[:, NT + t : NT + t + 1],
                    )
                    acc = opool.tile([128, D], F32, tag="acc")
                    nc.scalar.activation(
                        acc[:], s0, mybir.ActivationFunctionType.Copy,
                        scale=recip[:, t : t + 1],
                    )
                    nc.vector.tensor_add(acc[:], acc[:], o1[:])
                    nc.sync.dma_start(
                        out=out_partial[t * 128 : (t + 1) * 128, :], in_=acc[:]
                    )

    nc.finalize()
    return nc


def kernel(q, k, v, wq_w, wq_b, wk_w, wk_b, wv_w, wv_b, wo_w, wo_b, _profile=False):
    global _CACHED_NC
    q = np.asarray(q, np.float32)
    k = np.asarray(k, np.float32)
    v = np.asarray(v, np.float32)
    wq_w = np.asarray(wq_w, np.float32)
    wk_w = np.asarray(wk_w, np.float32)
    wv_w = np.asarray(wv_w, np.float32)
    wo_w = np.asarray(wo_w, np.float32)

    if _CACHED_NC is None:
        _CACHED_NC = _build_nc()
    nc = _CACHED_NC

    xT = {}
    for b in range(B):
        xT[("q", b)] = np.ascontiguousarray(q[b].T.astype(np.float16))
        xT[("k", b)] = np.ascontiguousarray(k[b].T.astype(np.float16))
        xT[("v", b)] = np.ascontiguousarray(v[b].T.astype(np.float16))

    in_maps = []
    for c in range(N_CORES):
        b, hp = divmod(c, 4)
        cs = slice(hp * D2, (hp + 1) * D2)
        in_maps.append(
            {
                "xqT": xT[("q", b)],
                "xkT": xT[("k", b)],
                "xvT": xT[("v", b)],
                "wq": np.ascontiguousarray(wq_w[:, cs].astype(np.float16)),
                "wk": np.ascontiguousarray(wk_w[:, cs].astype(np.float16)),
                "wv": np.ascontiguousarray(wv_w[:, cs].astype(np.float16)),
                "wo": np.ascontiguousarray(wo_w[cs, :].astype(np.float16)),
                "bq": np.ascontiguousarray(np.asarray(wq_b, np.float32)[cs, None]),
                "bk": np.ascontiguousarray(np.asarray(wk_b, np.float32)[cs, None]),
            }
        )

    kwargs = {}
    if _profile:
        import os

        os.makedirs("/tmp/bass_trace", exist_ok=True)
        kwargs = {"trace": True, "tmpdir": "/tmp/bass_trace"}
    res = run_bass_kernel_spmd(nc, in_maps, list(range(N_CORES)), **kwargs)

    attn = np.empty((B, H, S, S), np.float32)
    out = np.zeros((B, S, D), np.float32)
    for c in range(N_CORES):
        b, hp = divmod(c, 4)
        attn[b, 2 * hp : 2 * hp + 2] = res.results[c]["attn_out"]
        out[b] += res.results[c]["out_partial"]
    # fold v/o biases: softmax rows sum to 1 -> ctx += wv_b, out += wv_b@wo + wo_b
    out += (
        np.asarray(wv_b, np.float32) @ wo_w + np.asarray(wo_b, np.float32)
    )[None, None, :]

    if _profile:
        return (out, attn), res
    return out, attn


# revision 46
# speedup vs baseline: 1.1281x; 1.1281x over previous
"""Multi-head attention (B=2, S=2048, D=512, H=8) on 8 trn2 NeuronCores.

Sharding: data-parallel over batch (2) x tensor-parallel over head-pairs (4).
Core c handles batch c//4 and heads [2*(c%4), 2*(c%4)+1] (128 model dims).

Device kernel (SPMD, identical program, per-core inputs):
  inputs:  xqT/xkT/xvT [512,2048] (host-pretransposed), wq/wk/wv [512,128]
           (column slice), wo [128,512] (row slice), bq/bk [128,1]
  outputs: attn_out [2,2048,2048] (this core's two heads, softmaxed),
           out_partial [2048,512] (this core's contribution to out)

Host folds the v/o biases in afterwards: since each softmax row sums to 1,
ctx = attn@(vh + 1*bv^T) = attn@vh + bv, so out += bv @ wo_w + wo_b.
"""

import numpy as np

import concourse.bass as bass
import concourse.mybir as mybir
from concourse import bacc
from concourse.tile import TileContext
from concourse.bass_utils import run_bass_kernel_spmd

B, S, D = 2, 2048, 512
H, DEP = 8, 64
N_CORES = 8
HPC = 2          # heads per core
D2 = HPC * DEP   # 128 model dims per core
NT = S // 128    # 16 tiles of 128 rows
F32 = mybir.dt.float32
F16 = mybir.dt.float16

_CACHED_NC = None


def _build_nc():
    nc = bacc.Bacc(None, target_bir_lowering=False, debug=False)

    xqT = nc.declare_dram_parameter("xqT", [D, S], F16, isOutput=False)
    xkT = nc.declare_dram_parameter("xkT", [D, S], F16, isOutput=False)
    xvT = nc.declare_dram_parameter("xvT", [D, S], F16, isOutput=False)
    wq = nc.declare_dram_parameter("wq", [D, D2], F16, isOutput=False)
    wk = nc.declare_dram_parameter("wk", [D, D2], F16, isOutput=False)
    wv = nc.declare_dram_parameter("wv", [D, D2], F16, isOutput=False)
    wo = nc.declare_dram_parameter("wo", [D2, D], F16, isOutput=False)
    bq = nc.declare_dram_parameter("bq", [D2, 1], F32, isOutput=False)
    bk = nc.declare_dram_parameter("bk", [D2, 1], F32, isOutput=False)
    attn_out = nc.declare_dram_parameter("attn_out", [HPC, S, S], F32, isOutput=True)
    out_partial = nc.declare_dram_parameter("out_partial", [S, D], F32, isOutput=True)

    scale = 1.0 / np.sqrt(DEP)
    EXP = mybir.ActivationFunctionType.Exp

    with TileContext(nc) as tc:
        with tc.tile_pool(name="singles", bufs=1) as singles:
            # persistent SBUF tensors
            qhT = singles.tile([D2, S], F16, tag="qhT")    # [128(2h x 64d), 2048]
            khT = singles.tile([D2, S], F16, tag="khT")
            vh = singles.tile([128, S], F16, tag="vh")     # block st: [128(k),128(d)]
            ctxT = singles.tile([D2, S], F16, tag="ctxT")  # [128(d), 2048(q)] unnorm
            recip = singles.tile([128, HPC * NT], F32, tag="recip")  # col h*16+t
            x_v = singles.tile([128, 4, S], F16, tag="x_v")
            wq_sb = singles.tile([128, 4, D2], F16, tag="wq")
            wk_sb = singles.tile([128, 4, D2], F16, tag="wk")
            wv_sb = singles.tile([128, 4, D2], F16, tag="wv")
            wo_sb = singles.tile([D2, D], F16, tag="wo")
            bq_sb = singles.tile([D2, 1], F32, tag="bq")
            bk_sb = singles.tile([D2, 1], F32, tag="bk")

            nc.sync.dma_start(out=wq_sb[:], in_=wq.ap().rearrange("(c p) d -> p c d", p=128))
            nc.sync.dma_start(out=wk_sb[:], in_=wk.ap().rearrange("(c p) d -> p c d", p=128))
            nc.sync.dma_start(out=wv_sb[:], in_=wv.ap().rearrange("(c p) d -> p c d", p=128))
            nc.sync.dma_start(out=wo_sb[:], in_=wo.ap())
            nc.sync.dma_start(out=bq_sb[:], in_=bq.ap())
            nc.sync.dma_start(out=bk_sb[:], in_=bk.ap())

            # ---------------- stage 1: q/k projections ----------------
            # (v is loaded here but projected inside phase A)
            with (
                tc.tile_pool(name="xpool", bufs=2) as xpool,
                tc.tile_pool(name="psum1", bufs=2, space="PSUM") as psum1,
            ):
                for xT, w_sb, b_sb, outT in (
                    (xkT, wk_sb, bk_sb, khT),
                    (xqT, wq_sb, bq_sb, qhT),
                ):
                    x_sb = xpool.tile([128, 4, S], F16, tag="x")
                    nc.sync.dma_start(
                        out=x_sb[:], in_=xT.ap().rearrange("(c p) s -> p c s", p=128)
                    )
                    for ncx in range(4):
                        ps = psum1.tile([D2, 512], F32, tag="ps1")
                        for cc in range(4):
                            nc.tensor.matmul(
                                ps[:],
                                w_sb[:, cc, :],
                                x_sb[:, cc, ncx * 512 : (ncx + 1) * 512],
                                start=(cc == 0),
                                stop=(cc == 3),
                            )
                        nc.vector.tensor_scalar_add(
                            outT[:, ncx * 512 : (ncx + 1) * 512], ps[:], b_sb[:]
                        )
                nc.sync.dma_start(
                    out=x_v[:], in_=xvT.ap().rearrange("(c p) s -> p c s", p=128)
                )

            # ---------------- stage 2: attention ----------------
            with (
                tc.tile_pool(name="psum_s", bufs=3, space="PSUM") as pool_s,
                tc.tile_pool(name="psum_av", bufs=2, space="PSUM") as pool_av,
                tc.tile_pool(name="Ppool", bufs=3) as Ppool,
                tc.tile_pool(name="PTpool", bufs=2) as PTpool,
                tc.tile_pool(name="sumpool", bufs=4) as sumpool,
                tc.tile_pool(name="opool", bufs=2) as opool,
            ):
                PTs = {}

                def vh_group(g):
                    """Project v s-tiles 4g..4g+3 into vh (dense 16-MM burst)."""
                    ps_v = pool_av.tile([128, 512], F32, tag="av", name="ps_v")
                    for i in range(4):
                        st = 4 * g + i
                        for cc in range(4):
                            nc.tensor.matmul(
                                ps_v[:, i * 128 : (i + 1) * 128],
                                x_v[:, cc, st * 128 : (st + 1) * 128],
                                wv_sb[:, cc, :],
                                start=(cc == 0),
                                stop=(cc == 3),
                            )
                    nc.vector.tensor_copy(
                        out=vh[:, 4 * g * 128 : (4 * g + 4) * 128], in_=ps_v[:]
                    )

                def av_part(hT, qc, c):
                    """8-deep AV chain (kt=c*8..c*8+7) folded into ctxT."""
                    ps_av = pool_av.tile([DEP, 512], F32, tag="av", name="ps_av")
                    PTh = PTs[hT]
                    for j in range(8):
                        kt = c * 8 + j
                        nc.tensor.matmul(
                            ps_av[:],
                            vh[:, kt * 128 + hT * DEP : kt * 128 + (hT + 1) * DEP],
                            PTh[:, kt, qc * 512 : (qc + 1) * 512],
                            start=(j == 0),
                            stop=(j == 7),
                        )
                    ct = ctxT[hT * DEP : (hT + 1) * DEP, qc * 512 : (qc + 1) * 512]
                    if c == 0:
                        nc.vector.tensor_copy(out=ct, in_=ps_av[:])
                    else:
                        nc.vector.tensor_add(ct, ct, ps_av[:])

                for phase in range(2):
                    hP, hT = phase, 1 - phase
                    hsP = slice(hP * DEP, (hP + 1) * DEP)
                    hsT = slice(hT * DEP, (hT + 1) * DEP)
                    PTs[hT] = PTpool.tile([128, NT, S], F16, tag="PT", name="PT")
                    for t in range(NT):
                        # ---- PT stream (head hT, k-tile t) ----
                        for half in range(2):
                            ps_t = pool_s.tile([128, 1024], F32, tag="s", name="ps_t")
                            for ncx in range(2):
                                nc.tensor.matmul(
                                    ps_t[:, ncx * 512 : (ncx + 1) * 512],
                                    khT[hsT, t * 128 : (t + 1) * 128],
                                    qhT[hsT, half * 1024 + ncx * 512 : half * 1024 + (ncx + 1) * 512],
                                    start=True,
                                    stop=True,
                                )
                            nc.scalar.activation(
                                PTs[hT][:, t, half * 1024 : (half + 1) * 1024],
                                ps_t[:],
                                EXP,
                                scale=float(scale),
                            )
                        # ---- P stream (head hP, q-tile t), other row-group ----
                        P_sb = Ppool.tile([128, S], F32, tag="P")
                        sums = sumpool.tile([128, 2], F32, tag="sums")
                        for half in range(2):
                            ps_p = pool_s.tile([128, 1024], F32, tag="s", name="ps_p")
                            for ncx in range(2):
                                nc.tensor.matmul(
                                    ps_p[:, ncx * 512 : (ncx + 1) * 512],
                                    qhT[hsP, t * 128 : (t + 1) * 128],
                                    khT[hsP, half * 1024 + ncx * 512 : half * 1024 + (ncx + 1) * 512],
                                    start=True,
                                    stop=True,
                                )
                            nc.scalar.activation(
                                P_sb[:, half * 1024 : (half + 1) * 1024],
                                ps_p[:], EXP,
                                scale=float(scale),
                                accum_out=sums[:, half : half + 1],
                            )
                        rc = recip[:, hP * NT + t : hP * NT + t + 1]
                        ssum = sumpool.tile([128, 1], F32, tag="ssum")
                        nc.vector.tensor_add(ssum[:], sums[:, 0:1], sums[:, 1:2])
                        nc.vector.reciprocal(rc, ssum[:])
                        nc.vector.tensor_scalar_mul(P_sb[:], P_sb[:], rc)
                        nc.sync.dma_start(
                            out=attn_out[hP, t * 128 : (t + 1) * 128, :], in_=P_sb[:]
                        )
                        # ---- dense PE interleaves ----
                        if phase == 0 and t % 4 == 1:
                            vh_group(t // 4)          # t=1,5,9,13
                        if phase == 0 and t in (9, 11, 13, 15):
                            av_part(1, (t - 9) // 2, 0)
                        elif phase == 1 and t in (1, 3, 5, 7):
                            av_part(1, (t - 1) // 2, 1)
                        elif phase == 1 and t in (9, 11, 13, 15):
                            av_part(0, (t - 9) // 2, 0)

                # ---- tail: AV(h0) second half + output projection ----
                for t in range(NT):
                    if t % 4 == 0:
                        av_part(0, t // 4, 1)
                    ps_o = pool_s.tile([128, 1024], F32, tag="s", name="ps_o")
                    s1 = ps_o[:, 0:512]
                    s0 = ps_o[:, 512:1024]
                    nc.tensor.matmul(
                        s1, ctxT[DEP:, t * 128 : (t + 1) * 128], wo_sb[DEP:, :],
                        start=True, stop=True,
                    )
                    nc.tensor.matmul(
                        s0, ctxT[:DEP, t * 128 : (t + 1) * 128], wo_sb[:DEP, :],
                        start=True, stop=True,
                    )
                    o1 = opool.tile([128, D], F32, tag="o1")
                    nc.scalar.activation(
                        o1[:], s1, mybir.ActivationFunctionType.Copy,
                        scale=recip[:, NT + t : NT + t + 1],
                    )
                    acc = opool.tile([128, D], F32, tag="acc")
                    nc.scalar.activation(
                        acc[:], s0, mybir.ActivationFunctionType.Copy,
                        scale=recip[:, t : t + 1],
                    )
                    nc.vector.tensor_add(acc[:], acc[:], o1[:])
                    nc.sync.dma_start(
                        out=out_partial[t * 128 : (t + 1) * 128, :], in_=acc[:]
                    )

    nc.finalize()
    return nc


def kernel(q, k, v, wq_w, wq_b, wk_w, wk_b, wv_w, wv_b, wo_w, wo_b, _profile=False):
    global _CACHED_NC
    q = np.asarray(q, np.float32)
    k = np.asarray(k, np.float32)
    v = np.asarray(v, np.float32)
    wq_w = np.asarray(wq_w, np.float32)
    wk_w = np.asarray(wk_w, np.float32)
    wv_w = np.asarray(wv_w, np.float32)
    wo_w = np.asarray(wo_w, np.float32)

    if _CACHED_NC is None:
        _CACHED_NC = _build_nc()
    nc = _CACHED_NC

    xT = {}
    for b in range(B):
        xT[("q", b)] = np.ascontiguousarray(q[b].T.astype(np.float16))
        xT[("k", b)] = np.ascontiguousarray(k[b].T.astype(np.float16))
        xT[("v", b)] = np.ascontiguousarray(v[b].T.astype(np.float16))

    in_maps = []
    for c in range(N_CORES):
        b, hp = divmod(c, 4)
        cs = slice(hp * D2, (hp + 1) * D2)
        in_maps.append(
            {
                "xqT": xT[("q", b)],
                "xkT": xT[("k", b)],
                "xvT": xT[("v", b)],
                "wq": np.ascontiguousarray(wq_w[:, cs].astype(np.float16)),
                "wk": np.ascontiguousarray(wk_w[:, cs].astype(np.float16)),
                "wv": np.ascontiguousarray(wv_w[:, cs].astype(np.float16)),
                "wo": np.ascontiguousarray(wo_w[cs, :].astype(np.float16)),
                "bq": np.ascontiguousarray(np.asarray(wq_b, np.float32)[cs, None]),
                "bk": np.ascontiguousarray(np.asarray(wk_b, np.float32)[cs, None]),
            }
        )

    kwargs = {}
    if _profile:
        import os

        os.makedirs("/tmp/bass_trace", exist_ok=True)
        kwargs = {"trace": True, "tmpdir": "/tmp/bass_trace"}
    res = run_bass_kernel_spmd(nc, in_maps, list(range(N_CORES)), **kwargs)

    attn = np.empty((B, H, S, S), np.float32)
    out = np.zeros((B, S, D), np.float32)
    for c in range(N_CORES):
        b, hp = divmod(c, 4)
        attn[b, 2 * hp : 2 * hp + 2] = res.results[c]["attn_out"]
        out[b] += res.results[c]["out_partial"]
    # fold v/o biases: softmax rows sum to 1 -> ctx += wv_b, out += wv_b@wo + wo_b
    out += (
        np.asarray(wv_b, np.float32) @ wo_w + np.asarray(wo_b, np.float32)
    )[None, None, :]

    if _profile:
        return (out, attn), res
    return out, attn


# revision 47
# speedup vs baseline: 1.1772x; 1.0436x over previous
"""Multi-head attention (B=2, S=2048, D=512, H=8) on 8 trn2 NeuronCores.

Sharding: data-parallel over batch (2) x tensor-parallel over head-pairs (4).
Core c handles batch c//4 and heads [2*(c%4), 2*(c%4)+1] (128 model dims).

Device kernel (SPMD, identical program, per-core inputs):
  inputs:  xqT/xkT/xvT [512,2048] (host-pretransposed), wq/wk/wv [512,128]
           (column slice), wo [128,512] (row slice), bq/bk [128,1]
  outputs: attn_out [2,2048,2048] (this core's two heads, softmaxed),
           out_partial [2048,512] (this core's contribution to out)

Host folds the v/o biases in afterwards: since each softmax row sums to 1,
ctx = attn@(vh + 1*bv^T) = attn@vh + bv, so out += bv @ wo_w + wo_b.
"""

import numpy as np

import concourse.bass as bass
import concourse.mybir as mybir
from concourse import bacc
from concourse.tile import TileContext
from concourse.bass_utils import run_bass_kernel_spmd

B, S, D = 2, 2048, 512
H, DEP = 8, 64
N_CORES = 8
HPC = 2          # heads per core
D2 = HPC * DEP   # 128 model dims per core
NT = S // 128    # 16 tiles of 128 rows
F32 = mybir.dt.float32
F16 = mybir.dt.float16

_CACHED_NC = None


def _build_nc():
    nc = bacc.Bacc(None, target_bir_lowering=False, debug=False)

    xqT = nc.declare_dram_parameter("xqT", [D, S], F16, isOutput=False)
    xkT = nc.declare_dram_parameter("xkT", [D, S], F16, isOutput=False)
    xvT = nc.declare_dram_parameter("xvT", [D, S], F16, isOutput=False)
    wq = nc.declare_dram_parameter("wq", [D, D2], F16, isOutput=False)
    wk = nc.declare_dram_parameter("wk", [D, D2], F16, isOutput=False)
    wv = nc.declare_dram_parameter("wv", [D, D2], F16, isOutput=False)
    wo = nc.declare_dram_parameter("wo", [D2, D], F16, isOutput=False)
    bq = nc.declare_dram_parameter("bq", [D2, 1], F32, isOutput=False)
    bk = nc.declare_dram_parameter("bk", [D2, 1], F32, isOutput=False)
    attn_out = nc.declare_dram_parameter("attn_out", [HPC, S, S], F32, isOutput=True)
    out_partial = nc.declare_dram_parameter("out_partial", [S, D], F32, isOutput=True)

    scale = 1.0 / np.sqrt(DEP)
    EXP = mybir.ActivationFunctionType.Exp

    with TileContext(nc) as tc:
        with tc.tile_pool(name="singles", bufs=1) as singles:
            # persistent SBUF tensors
            qhT = singles.tile([D2, S], F16, tag="qhT")    # [128(2h x 64d), 2048]
            khT = singles.tile([D2, S], F16, tag="khT")
            vh = singles.tile([128, S], F16, tag="vh")     # block st: [128(k),128(d)]
            ctxT = singles.tile([D2, S], F16, tag="ctxT")  # [128(d), 2048(q)] unnorm
            recip = singles.tile([128, HPC * NT], F32, tag="recip")  # col h*16+t
            x_v = singles.tile([128, 4, S], F16, tag="x_v")
            wq_sb = singles.tile([128, 4, D2], F16, tag="wq")
            wk_sb = singles.tile([128, 4, D2], F16, tag="wk")
            wv_sb = singles.tile([128, 4, D2], F16, tag="wv")
            wo_sb = singles.tile([D2, D], F16, tag="wo")
            bq_sb = singles.tile([D2, 1], F32, tag="bq")
            bk_sb = singles.tile([D2, 1], F32, tag="bk")

            nc.sync.dma_start(out=wq_sb[:], in_=wq.ap().rearrange("(c p) d -> p c d", p=128))
            nc.sync.dma_start(out=wk_sb[:], in_=wk.ap().rearrange("(c p) d -> p c d", p=128))
            nc.sync.dma_start(out=wv_sb[:], in_=wv.ap().rearrange("(c p) d -> p c d", p=128))
            nc.sync.dma_start(out=wo_sb[:], in_=wo.ap())
            nc.sync.dma_start(out=bq_sb[:], in_=bq.ap())
            nc.sync.dma_start(out=bk_sb[:], in_=bk.ap())

            # ---------------- stage 1: q/k projections ----------------
            # (v is loaded here but projected inside phase A)
            with (
                tc.tile_pool(name="xpool", bufs=2) as xpool,
                tc.tile_pool(name="psum1", bufs=2, space="PSUM") as psum1,
            ):
                for xT, w_sb, b_sb, outT in (
                    (xkT, wk_sb, bk_sb, khT),
                    (xqT, wq_sb, bq_sb, qhT),
                ):
                    x_sb = xpool.tile([128, 4, S], F16, tag="x")
                    nc.sync.dma_start(
                        out=x_sb[:], in_=xT.ap().rearrange("(c p) s -> p c s", p=128)
                    )
                    for ncx in range(4):
                        ps = psum1.tile([D2, 512], F32, tag="ps1")
                        for cc in range(4):
                            nc.tensor.matmul(
                                ps[:],
                                w_sb[:, cc, :],
                                x_sb[:, cc, ncx * 512 : (ncx + 1) * 512],
                                start=(cc == 0),
                                stop=(cc == 3),
                            )
                        nc.vector.tensor_scalar_add(
                            outT[:, ncx * 512 : (ncx + 1) * 512], ps[:], b_sb[:]
                        )
                nc.sync.dma_start(
                    out=x_v[:], in_=xvT.ap().rearrange("(c p) s -> p c s", p=128)
                )

            # ---------------- stage 2: attention ----------------
            with (
                tc.tile_pool(name="psum_s", bufs=3, space="PSUM") as pool_s,
                tc.tile_pool(name="psum_av", bufs=2, space="PSUM") as pool_av,
                tc.tile_pool(name="Ppool", bufs=3) as Ppool,
                tc.tile_pool(name="PTpool", bufs=2) as PTpool,
                tc.tile_pool(name="sumpool", bufs=4) as sumpool,
                tc.tile_pool(name="opool", bufs=2) as opool,
            ):
                PTs = {}

                def vh_group(g):
                    """Project v s-tiles 4g..4g+3 into vh (dense 16-MM burst)."""
                    ps_v = pool_av.tile([128, 512], F32, tag="av", name="ps_v")
                    for i in range(4):
                        st = 4 * g + i
                        for cc in range(4):
                            nc.tensor.matmul(
                                ps_v[:, i * 128 : (i + 1) * 128],
                                x_v[:, cc, st * 128 : (st + 1) * 128],
                                wv_sb[:, cc, :],
                                start=(cc == 0),
                                stop=(cc == 3),
                            )
                    nc.vector.tensor_copy(
                        out=vh[:, 4 * g * 128 : (4 * g + 4) * 128], in_=ps_v[:]
                    )

                def av_part(hT, qc, c):
                    """8-deep AV chain (kt=c*8..c*8+7) folded into ctxT."""
                    ps_av = pool_av.tile([DEP, 512], F32, tag="av", name="ps_av")
                    PTh = PTs[hT]
                    for j in range(8):
                        kt = c * 8 + j
                        nc.tensor.matmul(
                            ps_av[:],
                            vh[:, kt * 128 + hT * DEP : kt * 128 + (hT + 1) * DEP],
                            PTh[:, kt, qc * 512 : (qc + 1) * 512],
                            start=(j == 0),
                            stop=(j == 7),
                        )
                    ct = ctxT[hT * DEP : (hT + 1) * DEP, qc * 512 : (qc + 1) * 512]
                    if c == 0:
                        nc.vector.tensor_copy(out=ct, in_=ps_av[:])
                    else:
                        nc.vector.tensor_add(ct, ct, ps_av[:])

                for phase in range(2):
                    hP, hT = phase, 1 - phase
                    hsP = slice(hP * DEP, (hP + 1) * DEP)
                    hsT = slice(hT * DEP, (hT + 1) * DEP)
                    PTs[hT] = PTpool.tile([128, NT, S], F16, tag="PT", name="PT")
                    def emit_pt(t):
                        for half in range(2):
                            ps_t = pool_s.tile([128, 1024], F32, tag="s", name="ps_t")
                            for ncx in range(2):
                                nc.tensor.matmul(
                                    ps_t[:, ncx * 512 : (ncx + 1) * 512],
                                    khT[hsT, t * 128 : (t + 1) * 128],
                                    qhT[hsT, half * 1024 + ncx * 512 : half * 1024 + (ncx + 1) * 512],
                                    start=True,
                                    stop=True,
                                )
                            nc.scalar.activation(
                                PTs[hT][:, t, half * 1024 : (half + 1) * 1024],
                                ps_t[:],
                                EXP,
                                scale=float(scale),
                            )

                    def emit_p(t):
                        P_sb = Ppool.tile([128, S], F32, tag="P", name="P_sb")
                        sums = sumpool.tile([128, 2], F32, tag="sums", name="sums")
                        for half in range(2):
                            ps_p = pool_s.tile([128, 1024], F32, tag="s", name="ps_p")
                            for ncx in range(2):
                                nc.tensor.matmul(
                                    ps_p[:, ncx * 512 : (ncx + 1) * 512],
                                    qhT[hsP, t * 128 : (t + 1) * 128],
                                    khT[hsP, half * 1024 + ncx * 512 : half * 1024 + (ncx + 1) * 512],
                                    start=True,
                                    stop=True,
                                )
                            nc.scalar.activation(
                                P_sb[:, half * 1024 : (half + 1) * 1024],
                                ps_p[:], EXP,
                                scale=float(scale),
                                accum_out=sums[:, half : half + 1],
                            )
                        rc = recip[:, hP * NT + t : hP * NT + t + 1]
                        ssum = sumpool.tile([128, 1], F32, tag="ssum", name="ssum")
                        nc.vector.tensor_add(ssum[:], sums[:, 0:1], sums[:, 1:2])
                        nc.vector.reciprocal(rc, ssum[:])
                        nc.vector.tensor_scalar_mul(P_sb[:], P_sb[:], rc)
                        nc.sync.dma_start(
                            out=attn_out[hP, t * 128 : (t + 1) * 128, :], in_=P_sb[:]
                        )

                    for t in range(NT):
                        # first two steps of phase A: P first (gated only on
                        # khT + qhT chunk 0, so ACT starts ~10us earlier)
                        if phase == 0 and t < 2:
                            emit_p(t)
                            emit_pt(t)
                        else:
                            emit_pt(t)
                            emit_p(t)
                        # ---- dense PE interleaves ----
                        if phase == 0 and t % 4 == 1:
                            vh_group(t // 4)          # t=1,5,9,13
                        if phase == 0 and t in (9, 11, 13, 15):
                            av_part(1, (t - 9) // 2, 0)
                        elif phase == 1 and t in (1, 3, 5, 7):
                            av_part(1, (t - 1) // 2, 1)
                        elif phase == 1 and t in (9, 11, 13, 15):
                            av_part(0, (t - 9) // 2, 0)

                # ---- tail: AV(h0) second half + output projection ----
                for t in range(NT):
                    if t % 4 == 0:
                        av_part(0, t // 4, 1)
                    ps_o = pool_s.tile([128, 1024], F32, tag="s", name="ps_o")
                    s1 = ps_o[:, 0:512]
                    s0 = ps_o[:, 512:1024]
                    nc.tensor.matmul(
                        s1, ctxT[DEP:, t * 128 : (t + 1) * 128], wo_sb[DEP:, :],
                        start=True, stop=True,
                    )
                    nc.tensor.matmul(
                        s0, ctxT[:DEP, t * 128 : (t + 1) * 128], wo_sb[:DEP, :],
                        start=True, stop=True,
                    )
                    o1 = opool.tile([128, D], F32, tag="o1")
                    nc.scalar.activation(
                        o1[:], s1, mybir.ActivationFunctionType.Copy,
                        scale=recip[:, NT + t : NT + t + 1],
                    )
                    acc = opool.tile([128, D], F32, tag="acc")
                    nc.scalar.activation(
                        acc[:], s0, mybir.ActivationFunctionType.Copy,
                        scale=recip[:, t : t + 1],
                    )
                    nc.vector.tensor_add(acc[:], acc[:], o1[:])
                    nc.sync.dma_start(
                        out=out_partial[t * 128 : (t + 1) * 128, :], in_=acc[:]
                    )

    nc.finalize()
    return nc


def kernel(q, k, v, wq_w, wq_b, wk_w, wk_b, wv_w, wv_b, wo_w, wo_b, _profile=False):
    global _CACHED_NC
    q = np.asarray(q, np.float32)
    k = np.asarray(k, np.float32)
    v = np.asarray(v, np.float32)
    wq_w = np.asarray(wq_w, np.float32)
    wk_w = np.asarray(wk_w, np.float32)
    wv_w = np.asarray(wv_w, np.float32)
    wo_w = np.asarray(wo_w, np.float32)

    if _CACHED_NC is None:
        _CACHED_NC = _build_nc()
    nc = _CACHED_NC

    xT = {}
    for b in range(B):
        xT[("q", b)] = np.ascontiguousarray(q[b].T.astype(np.float16))
        xT[("k", b)] = np.ascontiguousarray(k[b].T.astype(np.float16))
        xT[("v", b)] = np.ascontiguousarray(v[b].T.astype(np.float16))

    in_maps = []
    for c in range(N_CORES):
        b, hp = divmod(c, 4)
        cs = slice(hp * D2, (hp + 1) * D2)
        in_maps.append(
            {
                "xqT": xT[("q", b)],
                "xkT": xT[("k", b)],
                "xvT": xT[("v", b)],
                "wq": np.ascontiguousarray(wq_w[:, cs].astype(np.float16)),
                "wk": np.ascontiguousarray(wk_w[:, cs].astype(np.float16)),
                "wv": np.ascontiguousarray(wv_w[:, cs].astype(np.float16)),
                "wo": np.ascontiguousarray(wo_w[cs, :].astype(np.float16)),
                "bq": np.ascontiguousarray(np.asarray(wq_b, np.float32)[cs, None]),
                "bk": np.ascontiguousarray(np.asarray(wk_b, np.float32)[cs, None]),
            }
        )

    kwargs = {}
    if _profile:
        import os

        os.makedirs("/tmp/bass_trace", exist_ok=True)
        kwargs = {"trace": True, "tmpdir": "/tmp/bass_trace"}
    res = run_bass_kernel_spmd(nc, in_maps, list(range(N_CORES)), **kwargs)

    attn = np.empty((B, H, S, S), np.float32)
    out = np.zeros((B, S, D), np.float32)
    for c in range(N_CORES):
        b, hp = divmod(c, 4)
        attn[b, 2 * hp : 2 * hp + 2] = res.results[c]["attn_out"]
        out[b] += res.results[c]["out_partial"]
    # fold v/o biases: softmax rows sum to 1 -> ctx += wv_b, out += wv_b@wo + wo_b
    out += (
        np.asarray(wv_b, np.float32) @ wo_w + np.asarray(wo_b, np.float32)
    )[None, None, :]

    if _profile:
        return (out, attn), res
    return out, attn
